# revision 26
# baseline (speedup 1.0000x reference)
"""Multi-head causal attention (B=2, S=2048, D=1024, H=16, hd=64) on 8 TRN2
NeuronCores.

Sharding: tensor-parallel over heads - 2 heads per core. Each core computes
Q/K/V for its 2 heads over the full sequence, causal attention, and a partial
output projection (its 128 context features x Wo slice). Host sums the 8
fp16 partials in fp32 and adds the bias.

v2 structure (vs v1):
  - scores for the core's two heads run as concurrent row-tiled matmuls
    (hd=64 contraction -> PE rows 0-63 / 64-127), sharing one 2-bank PSUM
    tile so a single exp instruction covers both heads per key chunk
  - softmax reciprocal via DVE reciprocal_approx_fast on staged rowsum rows
    (no Ln/Exp ACT-table thrash); rowsums still ride a ones column in V
  - batch pipelining: batch 1's QKV projection passes + V transposes are
    spread through batch 0's attention loop as PE filler, as are the
    deferred out-projection slabs, keeping the PE HAM-warm throughout
  - fp16 partial outputs (halves the output DMA)
"""
import sys

for _p in ("/opt/trn_rl_repo",):
    if _p not in sys.path:
        sys.path.insert(0, _p)

import numpy as np

import concourse.bass as bass
import concourse.mybir as mybir
import concourse.tile as tile
from concourse import bacc
from concourse.bass_utils import run_bass_kernel_spmd

B, S, D = 2, 2048, 1024
H, HD = 16, 64
T = B * S
NCORES = 8
HPC = H // NCORES              # heads per core = 2
CF = HPC * HD                  # per-core ctx features = 128
QBLK = 512                     # query block width
NQB = S // QBLK                # 4 query blocks per batch
KCH = 128                      # key chunk
NFC = D // 128                 # contraction chunks for projections
NTB = S // 512                 # token chunks per batch for projections = 4
F16 = mybir.dt.float16
F32 = mybir.dt.float32
F32R = mybir.dt.float32r
AF = mybir.ActivationFunctionType
MUL = mybir.AluOpType.mult
ADD = mybir.AluOpType.add


def build_kernel():
    nc = bacc.Bacc()
    xT = nc.dram_tensor("xT", [D, T], F16, kind="ExternalInput")
    wq = nc.dram_tensor("wq", [128, D], F16, kind="ExternalInput")
    wk = nc.dram_tensor("wk", [128, D], F16, kind="ExternalInput")
    wv = nc.dram_tensor("wv", [128, D], F16, kind="ExternalInput")
    wo = nc.dram_tensor("wo", [CF, D], F16, kind="ExternalInput")
    tri = nc.dram_tensor("tri", [128, 128], F16, kind="ExternalInput")
    ide = nc.dram_tensor("ide", [128, 64], F16, kind="ExternalInput")
    indp = nc.dram_tensor("indp", [2, 128], F16, kind="ExternalInput")
    part = nc.dram_tensor("part", [T, D], F16, kind="ExternalOutput")

    with tile.TileContext(nc) as tc:
        with (
            tc.tile_pool(name="persist", bufs=1) as persist,
            tc.tile_pool(name="qkv_sb", bufs=1) as qkv_sb,
            tc.tile_pool(name="xp", bufs=1) as xp,
            tc.tile_pool(name="probs", bufs=4) as probs_pool,
            tc.tile_pool(name="ctxp", bufs=3) as ctx_pool,
            tc.tile_pool(name="outp", bufs=3) as out_pool,
            tc.tile_pool(name="ps_spair", bufs=2, space="PSUM") as ps_spair,
            tc.tile_pool(name="ps_cps", bufs=1, space="PSUM") as ps_cps,
            tc.tile_pool(name="ps_big", bufs=2, space="PSUM") as ps_big,
        ):
            # ---- weights / constants ----
            wq_sb = persist.tile([128, NFC, 128], F16, tag="wq")
            wk_sb = persist.tile([128, NFC, 128], F16, tag="wk")
            wv_sb = persist.tile([128, NFC, 128], F16, tag="wv")
            wo_sb = persist.tile([128, D], F16, tag="wo")
            tri_sb = persist.tile([128, 128], F16, tag="tri")
            ide_sb = persist.tile([128, 64], F16, tag="ide")
            indA_sb = persist.tile([1, 128], F16, tag="indA")
            indB_sb = persist.tile([1, 128], F16, tag="indB")
            nc.sync.dma_start(wq_sb[:, :, :], wq[:, :].rearrange("p (c m) -> p c m", c=NFC))
            nc.sync.dma_start(wk_sb[:, :, :], wk[:, :].rearrange("p (c m) -> p c m", c=NFC))
            nc.sync.dma_start(wv_sb[:, :, :], wv[:, :].rearrange("p (c m) -> p c m", c=NFC))
            w_sbs = {"q": wq_sb, "k": wk_sb, "v": wv_sb}

            # ---- persistent activations ----
            # per-batch Q/K/V transposed: [2*hd, S]
            qkvt = {
                (t, b): qkv_sb.tile([128, S], F16, tag=f"{t}t{b}", name=f"{t}t{b}")
                for t in "qkv" for b in range(B)
            }
            # V natural layout per (batch, head): [keys 128, kc, hd | ones]
            vp = {
                (b, h): qkv_sb.tile([128, S // KCH, HD + 1], F16, tag=f"vp{b}{h}", name=f"vp{b}{h}")
                for b in range(B) for h in range(HPC)
            }

            # ---- x tiles: [128, S] per (batch, f-chunk), two half DMAs
            # each so the first projection pass starts early ----
            xs = {}
            for b in range(B):
                for f in range(NFC):
                    xs[(b, f)] = xp.tile([128, S], F16, tag=f"x{b}{f}", name=f"x{b}{f}")
            # first projection pass's eight chunks as small DMAs spread
            # over four queues so the PE starts early
            qs = [nc.scalar, nc.gpsimd, nc.sync]
            for f in range(NFC):
                qs[f % 3].dma_start(
                    xs[(0, f)][:, 0:512], xT[f * 128:(f + 1) * 128, 0:512]
                )
            for b in range(B):
                for f in range(NFC):
                    lo = 512 if b == 0 else 0
                    nc.sync.dma_start(
                        xs[(b, f)][:, lo:S],
                        xT[f * 128:(f + 1) * 128, b * S + lo:(b + 1) * S],
                    )
                if b == 0:
                    nc.sync.dma_start(tri_sb[:, :], tri[:, :])
                    nc.sync.dma_start(ide_sb[:, :], ide[:, :])
            nc.sync.dma_start(wo_sb[:, :], wo[:, :])
            nc.sync.dma_start(indA_sb[:, :], indp[0:1, :])
            nc.sync.dma_start(indB_sb[:, :], indp[1:2, :])

            # ---------------- reusable emitters ----------------
            def emit_proj_pass(t, b, tb):
                """One projection pass: accumulate w_t.T @ x over NFC chunks."""
                ps = ps_big.tile([128, 512], F32, tag="big")
                for f in range(NFC):
                    nc.tensor.matmul(
                        ps[:, :], w_sbs[t][:, f, :],
                        xs[(b, f)][:, tb * 512:(tb + 1) * 512],
                        start=(f == 0), stop=(f == NFC - 1),
                    )
                sl = slice(tb * 512, (tb + 1) * 512)
                nc.vector.tensor_copy(qkvt[(t, b)][:, sl], ps[:, :])

            def emit_vtrans(b, h, g):
                """Transpose 4 key chunks of V into natural layout."""
                hp = slice(h * HD, (h + 1) * HD)
                pvt = ps_big.tile([128, 256], F16, tag="big")
                for j in range(4):
                    kc = 4 * g + j
                    nc.tensor.transpose(
                        pvt[:, j * 64:(j + 1) * 64],
                        qkvt[("v", b)][hp, kc * KCH:(kc + 1) * KCH],
                        ide_sb[hp, :],
                    )
                for j in range(4):
                    nc.vector.tensor_copy(
                        vp[(b, h)][:, 4 * g + j, 0:HD],
                        pvt[:, j * 64:(j + 1) * 64],
                    )

            def emit_vp_memset(b):
                for h in range(HPC):
                    nc.vector.memset(vp[(b, h)][:, :, :], 1.0)

            def emit_norm(rws16, ctx_sb):
                """Broadcast 1/rowsum to [128, QBLK] and scale ctx."""
                recb = ps_big.tile([128, 512], F32, tag="big")
                nc.tensor.matmul(
                    recb[:, :], indA_sb[:, :],
                    rws16[0:1, 0:QBLK],
                    start=True, stop=False,
                )
                nc.tensor.matmul(
                    recb[:, :], indB_sb[:, :],
                    rws16[0:1, QBLK:2 * QBLK],
                    start=False, stop=True,
                )
                nc.vector.tensor_tensor(ctx_sb[:, :], ctx_sb[:, :], recb[:, :], MUL)

            def emit_outproj_tch(ctx_sb, t0, tch):
                """[128 tokens, 1024 dims] of the output projection: two MMs,
                PSUM evacuation split across DVE and ACT, one contiguous DMA."""
                osb = out_pool.tile([128, D], F16, tag="o")
                for i in range(2):
                    ops = ps_big.tile([128, 512], F32, tag="big")
                    nc.tensor.matmul(
                        ops[:, :],
                        ctx_sb[:, tch * 128:(tch + 1) * 128],
                        wo_sb[:, i * 512:(i + 1) * 512],
                        start=True, stop=True,
                    )
                    if i == 0:
                        nc.vector.tensor_copy(osb[:, 0:512], ops[:, :])
                    else:
                        nc.scalar.copy(osb[:, 512:1024], ops[:, :])
                nc.sync.dma_start(
                    part[t0 + tch * 128:t0 + (tch + 1) * 128, :], osb[:, :]
                )

            # ---------------- filler unit queue ----------------
            # projection/vtrans/norm/outproj work is queued here and pumped
            # into the attention loops; flush_to() guarantees a block's
            # prerequisites are emitted before the block reads them
            fillers = []
            consumed = [0]
            debt = [0.0]  # emitted ACT-time minus emitted PE-time (ns)

            def pump():
                if fillers:
                    est, fn = fillers.pop(0)
                    fn()
                    consumed[0] += 1
                    debt[0] -= est

            def pump_balance():
                while fillers and debt[0] > 0:
                    pump()

            def flush_to(k):
                while fillers and consumed[0] < k:
                    pump()

            def flush():
                while fillers:
                    pump()

            emit_vp_memset(0)
            emit_vp_memset(1)
            prereq = {}
            for tb in range(NTB):
                for t in "qkv":
                    fillers.append(
                        (1750, lambda t=t, tb=tb: emit_proj_pass(t, 0, tb)))
                for h in range(HPC):
                    fillers.append(
                        (600, lambda h=h, g=tb: emit_vtrans(0, h, g)))
                prereq[(0, tb)] = len(fillers)
            for g in range(NTB):
                fillers.append((1750, lambda g=g: emit_proj_pass("v", 1, g)))
                for h in range(HPC):
                    fillers.append(
                        (600, lambda h=h, g=g: emit_vtrans(1, h, g)))
            for tb in range(NTB):
                fillers.append((1750, lambda tb=tb: emit_proj_pass("q", 1, tb)))
                fillers.append((1750, lambda tb=tb: emit_proj_pass("k", 1, tb)))
                prereq[(1, tb)] = len(fillers)

            # ================= attention =================
            def emit_attention(b):
                toff = b * S
                qt, kt = qkvt[("q", b)], qkvt[("k", b)]
                # batch 0 ascending (interleaves with its own projections),
                # batch 1 descending (smallest block last -> short tail)
                for qb in (range(NQB) if b == 0 else (3, 2, 1, 0)):
                    bi = b * NQB + qb
                    flush_to(prereq[(b, qb)])
                    q0 = qb * QBLK
                    nk = (q0 + QBLK) // KCH
                    cps = ps_cps.tile([HD + 1, HPC * QBLK], F32, tag="cps")
                    ctx_sb = ctx_pool.tile([128, QBLK], F16, tag="ctx")
                    pend = []  # [(probs, kc, off)] ctx trails scores by 2
                    for kc in range(nk):
                        off = max(0, kc * KCH - q0)
                        diag = kc * KCH >= q0
                        ksl = slice(kc * KCH, (kc + 1) * KCH)
                        spair = ps_spair.tile([128, HPC * QBLK], F32, tag="sp")
                        for h in range(HPC):
                            hp = slice(h * HD, (h + 1) * HD)
                            nc.tensor.matmul(
                                spair[:, h * QBLK + off:(h + 1) * QBLK],
                                kt[hp, ksl],
                                qt[hp, q0 + off:q0 + QBLK],
                                start=True, stop=True,
                            )
                        probs = probs_pool.tile([128, HPC * QBLK], F16, tag="p")
                        # one exp spans both heads; the unwritten strip
                        # [QBLK:QBLK+off] is exp'd too but never read
                        nc.scalar.activation(
                            probs[:, off:], spair[:, off:],
                            AF.Exp, bias=0.0, scale=0.125,
                        )
                        if diag:
                            for h in range(HPC):
                                nc.gpsimd.tensor_tensor(
                                    probs[:, h * QBLK + off:h * QBLK + off + KCH],
                                    probs[:, h * QBLK + off:h * QBLK + off + KCH],
                                    tri_sb[:, :],
                                    MUL,
                                )
                        pend.append((probs, kc, off))
                        if len(pend) > 2:
                            p_, k_, o_ = pend.pop(0)
                            for h in range(HPC):
                                nc.tensor.matmul(
                                    cps[:, h * QBLK + o_:(h + 1) * QBLK],
                                    vp[(b, h)][:, k_, :],
                                    p_[:, h * QBLK + o_:(h + 1) * QBLK],
                                    start=(k_ == 0), stop=(k_ == nk - 1),
                                )
                        debt[0] += (HPC * QBLK - off + 352) / 1.2
                        debt[0] -= 3 * (QBLK - off) / 2.4
                        pump_balance()
                    # drain pending chunks
                    for p_, k_, o_ in pend:
                        for h in range(HPC):
                            nc.tensor.matmul(
                                cps[:, h * QBLK + o_:(h + 1) * QBLK],
                                vp[(b, h)][:, k_, :],
                                p_[:, h * QBLK + o_:(h + 1) * QBLK],
                                start=(k_ == 0), stop=(k_ == nk - 1),
                            )
                    # evacuate ctx + rowsum rows
                    for h in range(HPC):
                        nc.vector.tensor_copy(
                            ctx_sb[h * HD:(h + 1) * HD, :],
                            cps[0:HD, h * QBLK:(h + 1) * QBLK],
                        )
                    # 1/rowsum straight from the PSUM rows (both heads in
                    # one free-dim pass)
                    rwsi = ctx_pool.tile([1, 2 * QBLK], F32, tag="rwsi", name="rwsi")
                    rws16 = ctx_pool.tile([1, 2 * QBLK], F16, tag="rws16", name="rws16")
                    rawr = ctx_pool.tile([1, 2 * QBLK], F32, tag="rawr", name="rawr")
                    nc.vector.tensor_copy(rawr[:, :], cps[HD:HD + 1, :])
                    nc.vector.reciprocal_approx_fast(rwsi[0:1, :], rawr[:, :])
                    nc.vector.tensor_copy(rws16[:, :], rwsi[:, :])
                    # queue normalization + out-projection for this block
                    t0 = toff + q0
                    fillers.append((1000, lambda r=rws16, c=ctx_sb: emit_norm(r, c)))
                    for tch in range(QBLK // 128):
                        fillers.append((
                            500,
                            lambda c=ctx_sb, t0=t0, tch=tch:
                            emit_outproj_tch(c, t0, tch),
                        ))

            emit_attention(0)
            emit_attention(1)
            # ---- tail: flush remaining fillers ----
            flush()
    nc.compile()
    return nc


_NC_CACHE = None


def _get_nc():
    global _NC_CACHE
    if _NC_CACHE is None:
        _NC_CACHE = build_kernel()
    return _NC_CACHE


def _prechunk(wT):
    # [D, 128] -> [128, NFC*128]: partition p holds chunk c at cols c*128..
    return np.ascontiguousarray(
        wT.reshape(NFC, 128, CF).transpose(1, 0, 2).reshape(128, NFC * CF)
    ).astype(np.float16)


def make_in_maps(x, Wq, Wk, Wv, Wo):
    xT = np.ascontiguousarray(x.reshape(T, D).T.astype(np.float16))
    tri = np.triu(np.ones((128, 128), dtype=np.float16))
    ide = np.concatenate([np.eye(64, dtype=np.float16)] * 2, axis=0)
    indp = np.zeros((2, 128), dtype=np.float32)
    indp[0, 0:64] = 1.0
    indp[1, 64:128] = 1.0
    in_maps = []
    for c in range(NCORES):
        rs = slice(c * CF, (c + 1) * CF)
        in_maps.append({
            "xT": xT,
            "wq": _prechunk(Wq[rs, :].T),
            "wk": _prechunk(Wk[rs, :].T),
            "wv": _prechunk(Wv[rs, :].T),
            "wo": np.ascontiguousarray(Wo[:, rs].T.astype(np.float16)),
            "tri": tri,
            "ide": ide,
            "indp": indp.astype(np.float16),
        })
    return in_maps


def kernel(x, Wq, Wk, Wv, Wo, bo):
    x = np.asarray(x, dtype=np.float32)
    Wq = np.asarray(Wq, dtype=np.float32)
    Wk = np.asarray(Wk, dtype=np.float32)
    Wv = np.asarray(Wv, dtype=np.float32)
    Wo = np.asarray(Wo, dtype=np.float32)
    bo = np.asarray(bo, dtype=np.float32)

    in_maps = make_in_maps(x, Wq, Wk, Wv, Wo)
    res = run_bass_kernel_spmd(_get_nc(), in_maps, core_ids=list(range(NCORES)))
    out = res.results[0]["part"].astype(np.float32)
    for c in range(1, NCORES):
        out += res.results[c]["part"].astype(np.float32)
    out += bo[None, :]
    return out.reshape(B, S, D)


# revision 27
# speedup vs baseline: 1.0402x; 1.0402x over previous
"""Multi-head causal attention (B=2, S=2048, D=1024, H=16, hd=64) on 8 TRN2
NeuronCores.

Sharding: tensor-parallel over heads - 2 heads per core. Each core computes
Q/K/V for its 2 heads over the full sequence, causal attention, and a partial
output projection (its 128 context features x Wo slice). Host sums the 8
fp16 partials in fp32 and adds the bias.

v2 structure (vs v1):
  - scores for the core's two heads run as concurrent row-tiled matmuls
    (hd=64 contraction -> PE rows 0-63 / 64-127), sharing one 2-bank PSUM
    tile so a single exp instruction covers both heads per key chunk
  - softmax reciprocal via DVE reciprocal_approx_fast on staged rowsum rows
    (no Ln/Exp ACT-table thrash); rowsums still ride a ones column in V
  - batch pipelining: batch 1's QKV projection passes + V transposes are
    spread through batch 0's attention loop as PE filler, as are the
    deferred out-projection slabs, keeping the PE HAM-warm throughout
  - fp16 partial outputs (halves the output DMA)
"""
import sys

for _p in ("/opt/trn_rl_repo",):
    if _p not in sys.path:
        sys.path.insert(0, _p)

import numpy as np

import concourse.bass as bass
import concourse.mybir as mybir
import concourse.tile as tile
from concourse import bacc
from concourse.bass_utils import run_bass_kernel_spmd

B, S, D = 2, 2048, 1024
H, HD = 16, 64
T = B * S
NCORES = 8
HPC = H // NCORES              # heads per core = 2
CF = HPC * HD                  # per-core ctx features = 128
QBLK = 512                     # query block width
NQB = S // QBLK                # 4 query blocks per batch
KCH = 128                      # key chunk
NFC = D // 128                 # contraction chunks for projections
NTB = S // 512                 # token chunks per batch for projections = 4
F16 = mybir.dt.float16
F32 = mybir.dt.float32
F32R = mybir.dt.float32r
AF = mybir.ActivationFunctionType
MUL = mybir.AluOpType.mult
ADD = mybir.AluOpType.add


def build_kernel():
    nc = bacc.Bacc()
    xT = nc.dram_tensor("xT", [D, T], F16, kind="ExternalInput")
    wq = nc.dram_tensor("wq", [128, D], F16, kind="ExternalInput")
    wk = nc.dram_tensor("wk", [128, D], F16, kind="ExternalInput")
    wv = nc.dram_tensor("wv", [128, D], F16, kind="ExternalInput")
    wo = nc.dram_tensor("wo", [CF, D], F16, kind="ExternalInput")
    tri = nc.dram_tensor("tri", [128, 128], F16, kind="ExternalInput")
    ide = nc.dram_tensor("ide", [128, 64], F16, kind="ExternalInput")
    indp = nc.dram_tensor("indp", [2, 128], F16, kind="ExternalInput")
    part = nc.dram_tensor("part", [T, D], F16, kind="ExternalOutput")

    with tile.TileContext(nc) as tc:
        with (
            tc.tile_pool(name="persist", bufs=1) as persist,
            tc.tile_pool(name="qkv_sb", bufs=1) as qkv_sb,
            tc.tile_pool(name="xp", bufs=1) as xp,
            tc.tile_pool(name="probs", bufs=4) as probs_pool,
            tc.tile_pool(name="ctxp", bufs=3) as ctx_pool,
            tc.tile_pool(name="outp", bufs=3) as out_pool,
            tc.tile_pool(name="ps_spair", bufs=2, space="PSUM") as ps_spair,
            tc.tile_pool(name="ps_cps", bufs=1, space="PSUM") as ps_cps,
            tc.tile_pool(name="ps_big", bufs=2, space="PSUM") as ps_big,
        ):
            # ---- weights / constants ----
            wq_sb = persist.tile([128, NFC, 128], F16, tag="wq")
            wk_sb = persist.tile([128, NFC, 128], F16, tag="wk")
            wv_sb = persist.tile([128, NFC, 128], F16, tag="wv")
            wo_sb = persist.tile([128, D], F16, tag="wo")
            tri_sb = persist.tile([128, 128], F16, tag="tri")
            ide_sb = persist.tile([128, 64], F16, tag="ide")
            indA_sb = persist.tile([1, 128], F16, tag="indA")
            indB_sb = persist.tile([1, 128], F16, tag="indB")
            nc.sync.dma_start(wq_sb[:, :, :], wq[:, :].rearrange("p (c m) -> p c m", c=NFC))
            nc.scalar.dma_start(wk_sb[:, :, :], wk[:, :].rearrange("p (c m) -> p c m", c=NFC))
            nc.gpsimd.dma_start(wv_sb[:, :, :], wv[:, :].rearrange("p (c m) -> p c m", c=NFC))
            w_sbs = {"q": wq_sb, "k": wk_sb, "v": wv_sb}

            # ---- persistent activations ----
            # per-batch Q/K/V transposed: [2*hd, S]
            qkvt = {
                (t, b): qkv_sb.tile([128, S], F16, tag=f"{t}t{b}", name=f"{t}t{b}")
                for t in "qkv" for b in range(B)
            }
            # V natural layout per (batch, head): [keys 128, kc, hd | ones]
            vp = {
                (b, h): qkv_sb.tile([128, S // KCH, HD + 1], F16, tag=f"vp{b}{h}", name=f"vp{b}{h}")
                for b in range(B) for h in range(HPC)
            }

            # ---- x tiles: [128, S] per (batch, f-chunk), two half DMAs
            # each so the first projection pass starts early ----
            xs = {}
            for b in range(B):
                for f in range(NFC):
                    xs[(b, f)] = xp.tile([128, S], F16, tag=f"x{b}{f}", name=f"x{b}{f}")
            # first projection pass's eight chunks as small DMAs spread
            # over four queues so the PE starts early
            qs = [nc.scalar, nc.gpsimd, nc.sync]
            for f in range(NFC):
                qs[f % 3].dma_start(
                    xs[(0, f)][:, 0:512], xT[f * 128:(f + 1) * 128, 0:512]
                )
            for b in range(B):
                for f in range(NFC):
                    lo = 512 if b == 0 else 0
                    nc.sync.dma_start(
                        xs[(b, f)][:, lo:S],
                        xT[f * 128:(f + 1) * 128, b * S + lo:(b + 1) * S],
                    )
                if b == 0:
                    nc.sync.dma_start(tri_sb[:, :], tri[:, :])
                    nc.sync.dma_start(ide_sb[:, :], ide[:, :])
            nc.sync.dma_start(wo_sb[:, :], wo[:, :])
            nc.sync.dma_start(indA_sb[:, :], indp[0:1, :])
            nc.sync.dma_start(indB_sb[:, :], indp[1:2, :])

            # ---------------- reusable emitters ----------------
            def emit_proj_pass(t, b, tb):
                """One projection pass: accumulate w_t.T @ x over NFC chunks."""
                ps = ps_big.tile([128, 512], F32, tag="big")
                for f in range(NFC):
                    nc.tensor.matmul(
                        ps[:, :], w_sbs[t][:, f, :],
                        xs[(b, f)][:, tb * 512:(tb + 1) * 512],
                        start=(f == 0), stop=(f == NFC - 1),
                    )
                sl = slice(tb * 512, (tb + 1) * 512)
                nc.vector.tensor_copy(qkvt[(t, b)][:, sl], ps[:, :])

            def emit_vtrans(b, h, g):
                """Transpose 4 key chunks of V into natural layout."""
                hp = slice(h * HD, (h + 1) * HD)
                pvt = ps_big.tile([128, 256], F16, tag="big")
                for j in range(4):
                    kc = 4 * g + j
                    nc.tensor.transpose(
                        pvt[:, j * 64:(j + 1) * 64],
                        qkvt[("v", b)][hp, kc * KCH:(kc + 1) * KCH],
                        ide_sb[hp, :],
                    )
                for j in range(4):
                    nc.vector.tensor_copy(
                        vp[(b, h)][:, 4 * g + j, 0:HD],
                        pvt[:, j * 64:(j + 1) * 64],
                    )

            def emit_vp_memset(b):
                for h in range(HPC):
                    nc.vector.memset(vp[(b, h)][:, :, :], 1.0)

            def emit_norm(rws16, ctx_sb):
                """Broadcast 1/rowsum to [128, QBLK] and scale ctx."""
                recb = ps_big.tile([128, 512], F32, tag="big")
                nc.tensor.matmul(
                    recb[:, :], indA_sb[:, :],
                    rws16[0:1, 0:QBLK],
                    start=True, stop=False,
                )
                nc.tensor.matmul(
                    recb[:, :], indB_sb[:, :],
                    rws16[0:1, QBLK:2 * QBLK],
                    start=False, stop=True,
                )
                nc.vector.tensor_tensor(ctx_sb[:, :], ctx_sb[:, :], recb[:, :], MUL)

            def emit_outproj_tch(ctx_sb, t0, tch):
                """[128 tokens, 1024 dims] of the output projection: two MMs,
                PSUM evacuation split across DVE and ACT, one contiguous DMA."""
                osb = out_pool.tile([128, D], F16, tag="o")
                for i in range(2):
                    ops = ps_big.tile([128, 512], F32, tag="big")
                    nc.tensor.matmul(
                        ops[:, :],
                        ctx_sb[:, tch * 128:(tch + 1) * 128],
                        wo_sb[:, i * 512:(i + 1) * 512],
                        start=True, stop=True,
                    )
                    if i == 0:
                        nc.vector.tensor_copy(osb[:, 0:512], ops[:, :])
                    else:
                        nc.scalar.copy(osb[:, 512:1024], ops[:, :])
                nc.sync.dma_start(
                    part[t0 + tch * 128:t0 + (tch + 1) * 128, :], osb[:, :]
                )

            # ---------------- filler unit queue ----------------
            # projection/vtrans/norm/outproj work is queued here and pumped
            # into the attention loops; flush_to() guarantees a block's
            # prerequisites are emitted before the block reads them
            fillers = []
            consumed = [0]

            def pump():
                if fillers:
                    est, fn = fillers.pop(0)
                    fn()
                    consumed[0] += 1

            def flush_to(k):
                while fillers and consumed[0] < k:
                    pump()

            def flush():
                while fillers:
                    pump()

            emit_vp_memset(0)
            emit_vp_memset(1)
            prereq = {}
            for tb in range(NTB):
                for t in "qkv":
                    fillers.append(
                        (1750, lambda t=t, tb=tb: emit_proj_pass(t, 0, tb)))
                for h in range(HPC):
                    fillers.append(
                        (600, lambda h=h, g=tb: emit_vtrans(0, h, g)))
                prereq[(0, tb)] = len(fillers)
            for g in range(NTB):
                fillers.append((1750, lambda g=g: emit_proj_pass("v", 1, g)))
                for h in range(HPC):
                    fillers.append(
                        (600, lambda h=h, g=g: emit_vtrans(1, h, g)))
            for tb in range(NTB):
                fillers.append((1750, lambda tb=tb: emit_proj_pass("q", 1, tb)))
                fillers.append((1750, lambda tb=tb: emit_proj_pass("k", 1, tb)))
                prereq[(1, tb)] = len(fillers)

            # ================= attention =================
            def emit_attention(b):
                toff = b * S
                qt, kt = qkvt[("q", b)], qkvt[("k", b)]
                # batch 0 ascending (interleaves with its own projections),
                # batch 1 descending (smallest block last -> short tail)
                for qb in (range(NQB) if b == 0 else (3, 2, 1, 0)):
                    bi = b * NQB + qb
                    flush_to(prereq[(b, qb)])
                    q0 = qb * QBLK
                    nk = (q0 + QBLK) // KCH
                    cps = ps_cps.tile([HD + 1, HPC * QBLK], F32, tag="cps")
                    ctx_sb = ctx_pool.tile([128, QBLK], F16, tag="ctx")
                    pend = []  # [(probs, kc, off)] ctx trails scores by 2
                    for kc in range(nk):
                        off = max(0, kc * KCH - q0)
                        diag = kc * KCH >= q0
                        ksl = slice(kc * KCH, (kc + 1) * KCH)
                        spair = ps_spair.tile([128, HPC * QBLK], F32, tag="sp")
                        for h in range(HPC):
                            hp = slice(h * HD, (h + 1) * HD)
                            nc.tensor.matmul(
                                spair[:, h * QBLK + off:(h + 1) * QBLK],
                                kt[hp, ksl],
                                qt[hp, q0 + off:q0 + QBLK],
                                start=True, stop=True,
                            )
                        probs = probs_pool.tile([128, HPC * QBLK], F16, tag="p")
                        # one exp spans both heads; the unwritten strip
                        # [QBLK:QBLK+off] is exp'd too but never read
                        nc.scalar.activation(
                            probs[:, off:], spair[:, off:],
                            AF.Exp, bias=0.0, scale=0.125,
                        )
                        if diag:
                            for h in range(HPC):
                                nc.gpsimd.tensor_tensor(
                                    probs[:, h * QBLK + off:h * QBLK + off + KCH],
                                    probs[:, h * QBLK + off:h * QBLK + off + KCH],
                                    tri_sb[:, :],
                                    MUL,
                                )
                        pend.append((probs, kc, off))
                        if len(pend) > 2:
                            p_, k_, o_ = pend.pop(0)
                            for h in range(HPC):
                                nc.tensor.matmul(
                                    cps[:, h * QBLK + o_:(h + 1) * QBLK],
                                    vp[(b, h)][:, k_, :],
                                    p_[:, h * QBLK + o_:(h + 1) * QBLK],
                                    start=(k_ == 0), stop=(k_ == nk - 1),
                                )
                        pump()
                        if kc % 2 == 1:
                            pump()
                    # drain pending chunks
                    for p_, k_, o_ in pend:
                        for h in range(HPC):
                            nc.tensor.matmul(
                                cps[:, h * QBLK + o_:(h + 1) * QBLK],
                                vp[(b, h)][:, k_, :],
                                p_[:, h * QBLK + o_:(h + 1) * QBLK],
                                start=(k_ == 0), stop=(k_ == nk - 1),
                            )
                    # evacuate ctx + rowsum rows
                    for h in range(HPC):
                        nc.vector.tensor_copy(
                            ctx_sb[h * HD:(h + 1) * HD, :],
                            cps[0:HD, h * QBLK:(h + 1) * QBLK],
                        )
                    # 1/rowsum straight from the PSUM rows (both heads in
                    # one free-dim pass)
                    rwsi = ctx_pool.tile([1, 2 * QBLK], F32, tag="rwsi", name="rwsi")
                    rws16 = ctx_pool.tile([1, 2 * QBLK], F16, tag="rws16", name="rws16")
                    rawr = ctx_pool.tile([1, 2 * QBLK], F32, tag="rawr", name="rawr")
                    nc.vector.tensor_copy(rawr[:, :], cps[HD:HD + 1, :])
                    nc.vector.reciprocal_approx_fast(rwsi[0:1, :], rawr[:, :])
                    nc.vector.tensor_copy(rws16[:, :], rwsi[:, :])
                    # queue normalization + out-projection for this block
                    t0 = toff + q0
                    fillers.append((1000, lambda r=rws16, c=ctx_sb: emit_norm(r, c)))
                    for tch in range(QBLK // 128):
                        fillers.append((
                            500,
                            lambda c=ctx_sb, t0=t0, tch=tch:
                            emit_outproj_tch(c, t0, tch),
                        ))

            emit_attention(0)
            emit_attention(1)
            # ---- tail: flush remaining fillers ----
            flush()
    nc.compile()
    return nc


_NC_CACHE = None


def _get_nc():
    global _NC_CACHE
    if _NC_CACHE is None:
        _NC_CACHE = build_kernel()
    return _NC_CACHE


def _prechunk(wT):
    # [D, 128] -> [128, NFC*128]: partition p holds chunk c at cols c*128..
    return np.ascontiguousarray(
        wT.reshape(NFC, 128, CF).transpose(1, 0, 2).reshape(128, NFC * CF)
    ).astype(np.float16)


def make_in_maps(x, Wq, Wk, Wv, Wo):
    xT = np.ascontiguousarray(x.reshape(T, D).T.astype(np.float16))
    tri = np.triu(np.ones((128, 128), dtype=np.float16))
    ide = np.concatenate([np.eye(64, dtype=np.float16)] * 2, axis=0)
    indp = np.zeros((2, 128), dtype=np.float32)
    indp[0, 0:64] = 1.0
    indp[1, 64:128] = 1.0
    in_maps = []
    for c in range(NCORES):
        rs = slice(c * CF, (c + 1) * CF)
        in_maps.append({
            "xT": xT,
            "wq": _prechunk(Wq[rs, :].T),
            "wk": _prechunk(Wk[rs, :].T),
            "wv": _prechunk(Wv[rs, :].T),
            "wo": np.ascontiguousarray(Wo[:, rs].T.astype(np.float16)),
            "tri": tri,
            "ide": ide,
            "indp": indp.astype(np.float16),
        })
    return in_maps


def kernel(x, Wq, Wk, Wv, Wo, bo):
    x = np.asarray(x, dtype=np.float32)
    Wq = np.asarray(Wq, dtype=np.float32)
    Wk = np.asarray(Wk, dtype=np.float32)
    Wv = np.asarray(Wv, dtype=np.float32)
    Wo = np.asarray(Wo, dtype=np.float32)
    bo = np.asarray(bo, dtype=np.float32)

    in_maps = make_in_maps(x, Wq, Wk, Wv, Wo)
    res = run_bass_kernel_spmd(_get_nc(), in_maps, core_ids=list(range(NCORES)))
    out = res.results[0]["part"].astype(np.float32)
    for c in range(1, NCORES):
        out += res.results[c]["part"].astype(np.float32)
    out += bo[None, :]
    return out.reshape(B, S, D)


# revision 28
# speedup vs baseline: 1.1261x; 1.0825x over previous
"""Multi-head causal attention (B=2, S=2048, D=1024, H=16, hd=64) on 8 TRN2
NeuronCores.

Sharding: tensor-parallel over heads - 2 heads per core. Each core computes
Q/K/V for its 2 heads over the full sequence, causal attention, and a partial
output projection (its 128 context features x Wo slice). Host sums the 8
fp16 partials in fp32 and adds the bias.

v2 structure (vs v1):
  - scores for the core's two heads run as concurrent row-tiled matmuls
    (hd=64 contraction -> PE rows 0-63 / 64-127), sharing one 2-bank PSUM
    tile so a single exp instruction covers both heads per key chunk
  - softmax reciprocal via DVE reciprocal_approx_fast on staged rowsum rows
    (no Ln/Exp ACT-table thrash); rowsums still ride a ones column in V
  - batch pipelining: batch 1's QKV projection passes + V transposes are
    spread through batch 0's attention loop as PE filler, as are the
    deferred out-projection slabs, keeping the PE HAM-warm throughout
  - fp16 partial outputs (halves the output DMA)
"""
import sys

for _p in ("/opt/trn_rl_repo",):
    if _p not in sys.path:
        sys.path.insert(0, _p)

import numpy as np

import concourse.bass as bass
import concourse.mybir as mybir
import concourse.tile as tile
from concourse import bacc
from concourse.bass_utils import run_bass_kernel_spmd

B, S, D = 2, 2048, 1024
H, HD = 16, 64
T = B * S
NCORES = 8
HPC = H // NCORES              # heads per core = 2
CF = HPC * HD                  # per-core ctx features = 128
QBLK = 512                     # query block width
NQB = S // QBLK                # 4 query blocks per batch
KCH = 128                      # key chunk
NFC = D // 128                 # contraction chunks for projections
NTB = S // 512                 # token chunks per batch for projections = 4
F16 = mybir.dt.float16
F32 = mybir.dt.float32
F32R = mybir.dt.float32r
AF = mybir.ActivationFunctionType
MUL = mybir.AluOpType.mult
ADD = mybir.AluOpType.add


def build_kernel():
    nc = bacc.Bacc()
    xT = nc.dram_tensor("xT", [D, T], F16, kind="ExternalInput")
    wq = nc.dram_tensor("wq", [128, D], F16, kind="ExternalInput")
    wk = nc.dram_tensor("wk", [128, D], F16, kind="ExternalInput")
    wv = nc.dram_tensor("wv", [128, D], F16, kind="ExternalInput")
    wo = nc.dram_tensor("wo", [CF, D], F16, kind="ExternalInput")
    tri = nc.dram_tensor("tri", [128, 128], F16, kind="ExternalInput")
    ide = nc.dram_tensor("ide", [128, 64], F16, kind="ExternalInput")
    indp = nc.dram_tensor("indp", [2, 128], F16, kind="ExternalInput")
    part = nc.dram_tensor("part", [T, D], F16, kind="ExternalOutput")

    with tile.TileContext(nc) as tc:
        with (
            tc.tile_pool(name="persist", bufs=1) as persist,
            tc.tile_pool(name="qkv_sb", bufs=1) as qkv_sb,
            tc.tile_pool(name="xp", bufs=1) as xp,
            tc.tile_pool(name="probs", bufs=6) as probs_pool,
            tc.tile_pool(name="ctxp", bufs=3) as ctx_pool,
            tc.tile_pool(name="outp", bufs=3) as out_pool,
            tc.tile_pool(name="ps_spair", bufs=2, space="PSUM") as ps_spair,
            tc.tile_pool(name="ps_cps", bufs=1, space="PSUM") as ps_cps,
            tc.tile_pool(name="ps_big", bufs=2, space="PSUM") as ps_big,
        ):
            # ---- weights / constants ----
            wq_sb = persist.tile([128, NFC, 128], F16, tag="wq")
            wk_sb = persist.tile([128, NFC, 128], F16, tag="wk")
            wv_sb = persist.tile([128, NFC, 128], F16, tag="wv")
            wo_sb = persist.tile([128, D], F16, tag="wo")
            tri_sb = persist.tile([128, 128], F16, tag="tri")
            ide_sb = persist.tile([128, 64], F16, tag="ide")
            indA_sb = persist.tile([1, 128], F16, tag="indA")
            indB_sb = persist.tile([1, 128], F16, tag="indB")
            nc.sync.dma_start(wq_sb[:, :, :], wq[:, :].rearrange("p (c m) -> p c m", c=NFC))
            nc.scalar.dma_start(wk_sb[:, :, :], wk[:, :].rearrange("p (c m) -> p c m", c=NFC))
            nc.gpsimd.dma_start(wv_sb[:, :, :], wv[:, :].rearrange("p (c m) -> p c m", c=NFC))
            w_sbs = {"q": wq_sb, "k": wk_sb, "v": wv_sb}

            # ---- persistent activations ----
            # per-batch Q/K/V transposed: [2*hd, S]
            qkvt = {
                (t, b): qkv_sb.tile([128, S], F16, tag=f"{t}t{b}", name=f"{t}t{b}")
                for t in "qkv" for b in range(B)
            }
            # V natural layout per (batch, head): [keys 128, kc, hd | ones]
            vp = {
                (b, h): qkv_sb.tile([128, S // KCH, HD + 1], F16, tag=f"vp{b}{h}", name=f"vp{b}{h}")
                for b in range(B) for h in range(HPC)
            }

            # ---- x tiles: [128, S] per (batch, f-chunk), two half DMAs
            # each so the first projection pass starts early ----
            xs = {}
            for b in range(B):
                for f in range(NFC):
                    xs[(b, f)] = xp.tile([128, S], F16, tag=f"x{b}{f}", name=f"x{b}{f}")
            # first projection pass's eight chunks as small DMAs spread
            # over four queues so the PE starts early
            qs = [nc.scalar, nc.gpsimd, nc.sync]
            for f in range(NFC):
                qs[f % 3].dma_start(
                    xs[(0, f)][:, 0:512], xT[f * 128:(f + 1) * 128, 0:512]
                )
            for b in range(B):
                for f in range(NFC):
                    lo = 512 if b == 0 else 0
                    nc.sync.dma_start(
                        xs[(b, f)][:, lo:S],
                        xT[f * 128:(f + 1) * 128, b * S + lo:(b + 1) * S],
                    )
                if b == 0:
                    nc.sync.dma_start(tri_sb[:, :], tri[:, :])
                    nc.sync.dma_start(ide_sb[:, :], ide[:, :])
            nc.sync.dma_start(wo_sb[:, :], wo[:, :])
            nc.sync.dma_start(indA_sb[:, :], indp[0:1, :])
            nc.sync.dma_start(indB_sb[:, :], indp[1:2, :])

            # ---------------- reusable emitters ----------------
            def emit_proj_pass(t, b, tb):
                """One projection pass: accumulate w_t.T @ x over NFC chunks."""
                ps = ps_big.tile([128, 512], F32, tag="big")
                for f in range(NFC):
                    nc.tensor.matmul(
                        ps[:, :], w_sbs[t][:, f, :],
                        xs[(b, f)][:, tb * 512:(tb + 1) * 512],
                        start=(f == 0), stop=(f == NFC - 1),
                    )
                sl = slice(tb * 512, (tb + 1) * 512)
                nc.vector.tensor_copy(qkvt[(t, b)][:, sl], ps[:, :])

            def emit_vtrans(b, h, g):
                """Transpose 4 key chunks of V into natural layout."""
                hp = slice(h * HD, (h + 1) * HD)
                pvt = ps_big.tile([128, 256], F16, tag="big")
                for j in range(4):
                    kc = 4 * g + j
                    nc.tensor.transpose(
                        pvt[:, j * 64:(j + 1) * 64],
                        qkvt[("v", b)][hp, kc * KCH:(kc + 1) * KCH],
                        ide_sb[hp, :],
                    )
                for j in range(4):
                    nc.vector.tensor_copy(
                        vp[(b, h)][:, 4 * g + j, 0:HD],
                        pvt[:, j * 64:(j + 1) * 64],
                    )

            def emit_vp_memset(b):
                for h in range(HPC):
                    nc.vector.memset(vp[(b, h)][:, :, :], 1.0)

            def emit_norm(rws16, ctx_sb):
                """Broadcast 1/rowsum to [128, QBLK] and scale ctx."""
                recb = ps_big.tile([128, 512], F32, tag="big")
                nc.tensor.matmul(
                    recb[:, :], indA_sb[:, :],
                    rws16[0:1, 0:QBLK],
                    start=True, stop=False,
                )
                nc.tensor.matmul(
                    recb[:, :], indB_sb[:, :],
                    rws16[0:1, QBLK:2 * QBLK],
                    start=False, stop=True,
                )
                nc.vector.tensor_tensor(ctx_sb[:, :], ctx_sb[:, :], recb[:, :], MUL)

            def emit_outproj_tch(ctx_sb, t0, tch):
                """[128 tokens, 1024 dims] of the output projection: two MMs,
                PSUM evacuation split across DVE and ACT, one contiguous DMA."""
                osb = out_pool.tile([128, D], F16, tag="o")
                for i in range(2):
                    ops = ps_big.tile([128, 512], F32, tag="big")
                    nc.tensor.matmul(
                        ops[:, :],
                        ctx_sb[:, tch * 128:(tch + 1) * 128],
                        wo_sb[:, i * 512:(i + 1) * 512],
                        start=True, stop=True,
                    )
                    if i == 0:
                        nc.vector.tensor_copy(osb[:, 0:512], ops[:, :])
                    else:
                        nc.scalar.copy(osb[:, 512:1024], ops[:, :])
                nc.sync.dma_start(
                    part[t0 + tch * 128:t0 + (tch + 1) * 128, :], osb[:, :]
                )

            # ---------------- filler unit queue ----------------
            # projection/vtrans/norm/outproj work is queued here and pumped
            # into the attention loops; flush_to() guarantees a block's
            # prerequisites are emitted before the block reads them
            fillers = []
            consumed = [0]

            def pump():
                if fillers:
                    est, fn = fillers.pop(0)
                    fn()
                    consumed[0] += 1

            def flush_to(k):
                while fillers and consumed[0] < k:
                    pump()

            def flush():
                while fillers:
                    pump()

            emit_vp_memset(0)
            emit_vp_memset(1)
            prereq = {}
            for tb in range(NTB):
                for t in "qkv":
                    fillers.append(
                        (1750, lambda t=t, tb=tb: emit_proj_pass(t, 0, tb)))
                for h in range(HPC):
                    fillers.append(
                        (600, lambda h=h, g=tb: emit_vtrans(0, h, g)))
                prereq[(0, tb)] = len(fillers)
            for g in range(NTB):
                fillers.append((1750, lambda g=g: emit_proj_pass("v", 1, g)))
                for h in range(HPC):
                    fillers.append(
                        (600, lambda h=h, g=g: emit_vtrans(1, h, g)))
            for tb in range(NTB):
                fillers.append((1750, lambda tb=tb: emit_proj_pass("q", 1, tb)))
                fillers.append((1750, lambda tb=tb: emit_proj_pass("k", 1, tb)))
                prereq[(1, tb)] = len(fillers)

            # ================= attention =================
            def emit_attention(b):
                toff = b * S
                qt, kt = qkvt[("q", b)], qkvt[("k", b)]
                for qb in range(NQB):
                    bi = b * NQB + qb
                    flush_to(prereq[(b, qb)])
                    q0 = qb * QBLK
                    nk = (q0 + QBLK) // KCH
                    cps = ps_cps.tile([HD + 1, HPC * QBLK], F32, tag="cps")
                    ctx_sb = ctx_pool.tile([128, QBLK], F16, tag="ctx")
                    pend = []  # [(probs, kc, off)] ctx trails scores by 2
                    for kc in range(nk):
                        off = max(0, kc * KCH - q0)
                        diag = kc * KCH >= q0
                        ksl = slice(kc * KCH, (kc + 1) * KCH)
                        spair = ps_spair.tile([128, HPC * QBLK], F32, tag="sp")
                        for h in range(HPC):
                            hp = slice(h * HD, (h + 1) * HD)
                            nc.tensor.matmul(
                                spair[:, h * QBLK + off:(h + 1) * QBLK],
                                kt[hp, ksl],
                                qt[hp, q0 + off:q0 + QBLK],
                                start=True, stop=True,
                            )
                        probs = probs_pool.tile([128, HPC * QBLK], F16, tag="p")
                        # one exp spans both heads; the unwritten strip
                        # [QBLK:QBLK+off] is exp'd too but never read
                        nc.scalar.activation(
                            probs[:, off:], spair[:, off:],
                            AF.Exp, bias=0.0, scale=0.125,
                        )
                        if diag:
                            for h in range(HPC):
                                nc.gpsimd.tensor_tensor(
                                    probs[:, h * QBLK + off:h * QBLK + off + KCH],
                                    probs[:, h * QBLK + off:h * QBLK + off + KCH],
                                    tri_sb[:, :],
                                    MUL,
                                )
                        pend.append((probs, kc, off))
                        if len(pend) > 3:
                            p_, k_, o_ = pend.pop(0)
                            for h in range(HPC):
                                nc.tensor.matmul(
                                    cps[:, h * QBLK + o_:(h + 1) * QBLK],
                                    vp[(b, h)][:, k_, :],
                                    p_[:, h * QBLK + o_:(h + 1) * QBLK],
                                    start=(k_ == 0), stop=(k_ == nk - 1),
                                )
                        pump()
                        if kc % 2 == 1:
                            pump()
                    # drain pending chunks
                    for p_, k_, o_ in pend:
                        for h in range(HPC):
                            nc.tensor.matmul(
                                cps[:, h * QBLK + o_:(h + 1) * QBLK],
                                vp[(b, h)][:, k_, :],
                                p_[:, h * QBLK + o_:(h + 1) * QBLK],
                                start=(k_ == 0), stop=(k_ == nk - 1),
                            )
                    # evacuate ctx + rowsum rows
                    for h in range(HPC):
                        nc.vector.tensor_copy(
                            ctx_sb[h * HD:(h + 1) * HD, :],
                            cps[0:HD, h * QBLK:(h + 1) * QBLK],
                        )
                    # 1/rowsum straight from the PSUM rows (both heads in
                    # one free-dim pass)
                    rwsi = ctx_pool.tile([1, 2 * QBLK], F32, tag="rwsi", name="rwsi")
                    rws16 = ctx_pool.tile([1, 2 * QBLK], F16, tag="rws16", name="rws16")
                    rawr = ctx_pool.tile([1, 2 * QBLK], F32, tag="rawr", name="rawr")
                    nc.vector.tensor_copy(rawr[:, :], cps[HD:HD + 1, :])
                    nc.vector.reciprocal_approx_fast(rwsi[0:1, :], rawr[:, :])
                    nc.vector.tensor_copy(rws16[:, :], rwsi[:, :])
                    # queue normalization + out-projection for this block
                    t0 = toff + q0
                    fillers.append((1000, lambda r=rws16, c=ctx_sb: emit_norm(r, c)))
                    for tch in range(QBLK // 128):
                        fillers.append((
                            500,
                            lambda c=ctx_sb, t0=t0, tch=tch:
                            emit_outproj_tch(c, t0, tch),
                        ))

            emit_attention(0)
            emit_attention(1)
            # ---- tail: flush remaining fillers ----
            flush()
    nc.compile()
    return nc


_NC_CACHE = None


def _get_nc():
    global _NC_CACHE
    if _NC_CACHE is None:
        _NC_CACHE = build_kernel()
    return _NC_CACHE


def _prechunk(wT):
    # [D, 128] -> [128, NFC*128]: partition p holds chunk c at cols c*128..
    return np.ascontiguousarray(
        wT.reshape(NFC, 128, CF).transpose(1, 0, 2).reshape(128, NFC * CF)
    ).astype(np.float16)


def make_in_maps(x, Wq, Wk, Wv, Wo):
    xT = np.ascontiguousarray(x.reshape(T, D).T.astype(np.float16))
    tri = np.triu(np.ones((128, 128), dtype=np.float16))
    ide = np.concatenate([np.eye(64, dtype=np.float16)] * 2, axis=0)
    indp = np.zeros((2, 128), dtype=np.float32)
    indp[0, 0:64] = 1.0
    indp[1, 64:128] = 1.0
    in_maps = []
    for c in range(NCORES):
        rs = slice(c * CF, (c + 1) * CF)
        in_maps.append({
            "xT": xT,
            "wq": _prechunk(Wq[rs, :].T),
            "wk": _prechunk(Wk[rs, :].T),
            "wv": _prechunk(Wv[rs, :].T),
            "wo": np.ascontiguousarray(Wo[:, rs].T.astype(np.float16)),
            "tri": tri,
            "ide": ide,
            "indp": indp.astype(np.float16),
        })
    return in_maps


def kernel(x, Wq, Wk, Wv, Wo, bo):
    x = np.asarray(x, dtype=np.float32)
    Wq = np.asarray(Wq, dtype=np.float32)
    Wk = np.asarray(Wk, dtype=np.float32)
    Wv = np.asarray(Wv, dtype=np.float32)
    Wo = np.asarray(Wo, dtype=np.float32)
    bo = np.asarray(bo, dtype=np.float32)

    in_maps = make_in_maps(x, Wq, Wk, Wv, Wo)
    res = run_bass_kernel_spmd(_get_nc(), in_maps, core_ids=list(range(NCORES)))
    out = res.results[0]["part"].astype(np.float32)
    for c in range(1, NCORES):
        out += res.results[c]["part"].astype(np.float32)
    out += bo[None, :]
    return out.reshape(B, S, D)
